# revision 3
# baseline (speedup 1.0000x reference)
"""GQA attention layer for Trainium2, tensor-parallel over kv-heads on 8 NeuronCores.

Problem: x:(1,2048,2048) f32, causal mask; q/k/v/o projections with
NUM_HEADS=32, NUM_KV_HEADS=8, HEAD_DIM=128, GROUP=4.

Sharding: core c owns kv-head c and its 4 query heads (columns 4c*128..(4c+4)*128
of wq, rows of wo). Each core computes a partial y_c = attnout_c @ wo_c; the host
sums the 8 partials and adds bo.

Dataflow on each core (all "transposed" so no on-chip transposes of the big
probability matrix are ever needed):
  qT[d,i] = wq_c.T(h-major) accumulation:  matmul(lhsT=wq_kt, rhs=xT_kt)
  kT[d,j], vT[d,j] likewise;  v[j,d] via 16 PE transposes of vT.
  sT[j,i] = matmul(lhsT=kT_jtile, rhs=qT_chunk)   (contraction = head_dim, 1 mm)
  e = exp(sT * 1/sqrt(d))  on ACT (scale folded into qT drain), causal-masked
      with gpsimd.affine_select on the 4 diagonal j-tiles of each i-chunk.
  colsum[1,i] += ones.T @ e  (PE);  attnoutT[d,i] += v_jtile.T(=v) @ e (PE)
  recip = 1/colsum (DVE);  broadcast to 128 partitions with a k=1 PE matmul;
  aoT = avpsum * recip (DVE drain, bf16)
  y[i,hid] += aoT_head_tile.T @ wo_head  (4 head k-tiles), f32 out, DMA to HBM.

Causality: for i-chunk c (512 wide) only j-tiles 0..4c+3 are computed.
"""

import math

import numpy as np
import ml_dtypes

HIDDEN = 2048
HEAD_DIM = 128
NUM_HEADS = 32
NUM_KV = 8
GROUP = NUM_HEADS // NUM_KV
S = 2048
NCORES = 8
CH = 512                      # i-chunk width
NCH = S // CH                 # 4 i-chunks
KT = HIDDEN // 128            # 16 contraction tiles over hidden
NJT = S // 128                # 16 j-tiles
INV_SQRT_D = 1.0 / math.sqrt(HEAD_DIM)

# Module-level knobs for test.py (the grading harness uses the defaults).
TRACE = False
LAST_EXEC_NS = None
LAST_RESULTS = None

_PROG_CACHE = {}


def _build(mode):
    """mode: 'causal' (skip upper blocks, affine_select diag), 'full' (all-ones
    mask), 'generic' (multiplicative bf16 mask tiles from HBM)."""
    import concourse.bacc as bacc
    import concourse.tile as tile
    import concourse.mybir as mybir
    from concourse.masks import make_identity

    f32 = mybir.dt.float32
    bf16 = mybir.dt.bfloat16
    Ident = mybir.ActivationFunctionType.Identity
    Exp = mybir.ActivationFunctionType.Exp

    nc = bacc.Bacc(None, target_bir_lowering=False)

    xT_d = nc.dram_tensor("xT", [HIDDEN, S], bf16, kind="ExternalInput")
    wq_d = nc.dram_tensor("wq", [HIDDEN, GROUP * HEAD_DIM], bf16, kind="ExternalInput")
    wk_d = nc.dram_tensor("wk", [HIDDEN, HEAD_DIM], bf16, kind="ExternalInput")
    wv_d = nc.dram_tensor("wv", [HIDDEN, HEAD_DIM], bf16, kind="ExternalInput")
    wo_d = nc.dram_tensor("wo", [GROUP * HEAD_DIM, HIDDEN], bf16, kind="ExternalInput")
    bias_d = nc.dram_tensor("biasp", [128, 6], f32, kind="ExternalInput")
    if mode == "generic":
        mk_d = nc.dram_tensor("maskT", [S, S], bf16, kind="ExternalInput")
    y_d = nc.dram_tensor("y", [S, HIDDEN], f32, kind="ExternalOutput")

    def nblocks(c):
        return 4 * c + 4 if mode == "causal" else NJT

    with tile.TileContext(nc) as tc:
        with (
            tc.tile_pool(name="consts", bufs=1) as consts,
            tc.tile_pool(name="xw", bufs=1) as xw,
            tc.tile_pool(name="proj", bufs=1) as proj,
            tc.tile_pool(name="epool", bufs=6) as epool,
            tc.tile_pool(name="rpool", bufs=2) as rpool,
            tc.tile_pool(name="ypool", bufs=3) as ypool,
            tc.tile_pool(name="pp", bufs=2, space="PSUM") as pp,
            tc.tile_pool(name="spp", bufs=2, space="PSUM") as spp,
            tc.tile_pool(name="avp", bufs=2, space="PSUM") as avp,
            tc.tile_pool(name="csp", bufs=1, space="PSUM") as csp,
        ):
            # ---- constants ----
            ident = consts.tile([128, 128], bf16, tag="ident", name="ident")
            make_identity(nc, ident)
            ones_col = consts.tile([128, 1], bf16, tag="ones_col", name="ones_col")
            nc.vector.memset(ones_col, 1.0)
            ones_row = consts.tile([1, 128], f32, tag="ones_row", name="ones_row")
            nc.vector.memset(ones_row, 1.0)
            bias_sb = consts.tile([128, 6], f32, tag="bias", name="bias_sb")
            nc.sync.dma_start(out=bias_sb, in_=bias_d[:, :])

            # ---- input loads (k-tile-major so first proj block starts early) ----
            x_sb, wk_sb, wv_sb, wq_sb, wo_sb = [], [], [], [], []
            for kt in range(KT):
                wkt = xw.tile([128, HEAD_DIM], bf16, tag=f"wk{kt}", name=f"wk{kt}")
                nc.sync.dma_start(out=wkt, in_=wk_d[kt * 128:(kt + 1) * 128, :])
                wk_sb.append(wkt)
                xt = xw.tile([128, S], bf16, tag=f"x{kt}", name=f"x{kt}")
                nc.sync.dma_start(out=xt, in_=xT_d[kt * 128:(kt + 1) * 128, :])
                x_sb.append(xt)
            for kt in range(KT):
                wvt = xw.tile([128, HEAD_DIM], bf16, tag=f"wv{kt}", name=f"wv{kt}")
                nc.sync.dma_start(out=wvt, in_=wv_d[kt * 128:(kt + 1) * 128, :])
                wv_sb.append(wvt)
            for kt in range(KT):
                wqt = xw.tile([128, GROUP * HEAD_DIM], bf16, tag=f"wq{kt}", name=f"wq{kt}")
                nc.sync.dma_start(out=wqt, in_=wq_d[kt * 128:(kt + 1) * 128, :])
                wq_sb.append(wqt)
            for h in range(GROUP):
                wot = xw.tile([128, HIDDEN], bf16, tag=f"wo{h}", name=f"wo{h}")
                nc.sync.dma_start(out=wot, in_=wo_d[h * 128:(h + 1) * 128, :])
                wo_sb.append(wot)

            # ---- K projection: kT[d, j] per j-chunk ----
            kT_c = []
            for c in range(NCH):
                ps = pp.tile([128, CH], f32, tag="pp", name=f"psk{c}")
                for kt in range(KT):
                    nc.tensor.matmul(ps, lhsT=wk_sb[kt],
                                     rhs=x_sb[kt][:, c * CH:(c + 1) * CH],
                                     start=(kt == 0), stop=(kt == KT - 1))
                kt_t = proj.tile([128, CH], bf16, tag=f"kT{c}", name=f"kT{c}")
                nc.scalar.activation(kt_t, ps, Ident, bias=bias_sb[:, 4:5])
                kT_c.append(kt_t)

            # ---- V projection (vT) + PE transpose to v[j, d] ----
            vT_c = []
            for c in range(NCH):
                ps = pp.tile([128, CH], f32, tag="pp", name=f"psv{c}")
                for kt in range(KT):
                    nc.tensor.matmul(ps, lhsT=wv_sb[kt],
                                     rhs=x_sb[kt][:, c * CH:(c + 1) * CH],
                                     start=(kt == 0), stop=(kt == KT - 1))
                vt_t = proj.tile([128, CH], bf16, tag=f"vT{c}", name=f"vT{c}")
                nc.scalar.activation(vt_t, ps, Ident, bias=bias_sb[:, 5:6])
                vT_c.append(vt_t)
            v_sb = []
            for b in range(NJT):
                tp = spp.tile([128, 128], bf16, tag="s", name=f"tp{b}")
                nc.tensor.transpose(
                    tp, vT_c[b // 4][:, (b % 4) * 128:(b % 4 + 1) * 128], ident)
                vt = proj.tile([128, 128], bf16, tag=f"v{b}", name=f"v{b}")
                nc.vector.tensor_copy(vt, tp)
                v_sb.append(vt)

            # ---- Q projection: qT[(h, c)]; fold 1/sqrt(d) + scaled bias ----
            qT = {}
            for c in range(NCH):
                for h in range(GROUP):
                    ps = pp.tile([128, CH], f32, tag="pp", name=f"psq{h}_{c}")
                    for kt in range(KT):
                        nc.tensor.matmul(
                            ps, lhsT=wq_sb[kt][:, h * 128:(h + 1) * 128],
                            rhs=x_sb[kt][:, c * CH:(c + 1) * CH],
                            start=(kt == 0), stop=(kt == KT - 1))
                    qt_t = proj.tile([128, CH], bf16, tag=f"q{h}_{c}", name=f"q{h}_{c}")
                    nc.scalar.activation(qt_t, ps, Ident,
                                         bias=bias_sb[:, h:h + 1], scale=INV_SQRT_D)
                    qT[(h, c)] = qt_t

            # ---- generic-mode mask tiles (per chunk, shared across heads) ----
            mask_sb = {}

            # ---- attention + output projection, chunk-major ----
            aoT = {}
            for c in range(NCH):
                nb = nblocks(c)
                if mode == "generic":
                    for b in range(nb):
                        mt = proj.tile([128, CH], bf16, tag=f"m{b}", name=f"m{b}_{c}")
                        nc.sync.dma_start(
                            out=mt,
                            in_=mk_d[b * 128:(b + 1) * 128, c * CH:(c + 1) * CH])
                        mask_sb[b] = mt
                for h in range(GROUP):
                    av = avp.tile([128, CH], f32, tag="av", name=f"av{h}_{c}")
                    cs = csp.tile([1, CH], f32, tag="cs", name=f"cs{h}_{c}")
                    for b in range(nb):
                        sp_t = spp.tile([128, CH], f32, tag="s", name=f"s{h}_{c}_{b}")
                        nc.tensor.matmul(
                            sp_t, lhsT=kT_c[b // 4][:, (b % 4) * 128:(b % 4 + 1) * 128],
                            rhs=qT[(h, c)], start=True, stop=True)
                        e = epool.tile([128, CH], bf16, tag="e", name=f"e{h}_{c}_{b}")
                        nc.scalar.activation(e, sp_t, Exp)
                        if mode == "causal" and b >= 4 * c:
                            dd = b - 4 * c
                            nc.gpsimd.affine_select(
                                out=e, in_=e, pattern=[[1, CH]],
                                compare_op=mybir.AluOpType.is_ge,
                                fill=0.0, base=-128 * dd, channel_multiplier=-1)
                        elif mode == "generic":
                            nc.vector.tensor_mul(e, e, mask_sb[b])
                        nc.tensor.matmul(cs, lhsT=ones_col, rhs=e,
                                         start=(b == 0), stop=(b == nb - 1),
                                         skip_group_check=True)
                        nc.tensor.matmul(av, lhsT=v_sb[b], rhs=e,
                                         start=(b == 0), stop=(b == nb - 1),
                                         skip_group_check=True)
                    recip = rpool.tile([1, CH], f32, tag="recip", name=f"rc{h}_{c}")
                    nc.vector.reciprocal(recip, cs)
                    rb_ps = spp.tile([128, CH], f32, tag="s", name=f"rbp{h}_{c}")
                    nc.tensor.matmul(rb_ps, lhsT=ones_row, rhs=recip,
                                     start=True, stop=True)
                    rb = rpool.tile([128, CH], f32, tag="rb", name=f"rb{h}_{c}")
                    nc.vector.tensor_copy(rb, rb_ps)
                    ao = proj.tile([128, CH], bf16, tag=f"ao{h}_{c}", name=f"ao{h}_{c}")
                    nc.vector.tensor_mul(ao, av, rb)
                    aoT[(h, c)] = ao
                # y projection for this chunk
                for it in range(CH // 128):
                    for nh in range(NCH):
                        yp = pp.tile([128, CH], f32, tag="pp", name=f"yp{c}_{it}_{nh}")
                        for h in range(GROUP):
                            nc.tensor.matmul(
                                yp, lhsT=aoT[(h, c)][:, it * 128:(it + 1) * 128],
                                rhs=wo_sb[h][:, nh * CH:(nh + 1) * CH],
                                start=(h == 0), stop=(h == GROUP - 1))
                        ysb = ypool.tile([128, CH], f32, tag="y", name=f"y{c}_{it}_{nh}")
                        nc.vector.tensor_copy(ysb, yp)
                        nc.sync.dma_start(
                            out=y_d[c * CH + it * 128: c * CH + (it + 1) * 128,
                                    nh * CH:(nh + 1) * CH],
                            in_=ysb)
    nc.finalize()
    return nc


def _get_prog(mode):
    if mode not in _PROG_CACHE:
        _PROG_CACHE[mode] = _build(mode)
    return _PROG_CACHE[mode]


def kernel(x, mask, wq, bq, wk, bk, wv, bv, wo, bo):
    global LAST_EXEC_NS, LAST_RESULTS
    from concourse.bass_utils import run_bass_kernel_spmd

    bf = ml_dtypes.bfloat16
    x = np.asarray(x, dtype=np.float32)
    mask = np.asarray(mask)
    wq = np.asarray(wq, dtype=np.float32)
    bq = np.asarray(bq, dtype=np.float32)
    wk = np.asarray(wk, dtype=np.float32)
    bk = np.asarray(bk, dtype=np.float32)
    wv = np.asarray(wv, dtype=np.float32)
    bv = np.asarray(bv, dtype=np.float32)
    wo = np.asarray(wo, dtype=np.float32)
    bo = np.asarray(bo, dtype=np.float32)

    m2 = mask[0, 0]
    if np.array_equal(m2 != 0, np.tril(np.ones((S, S), dtype=bool))):
        mode = "causal"
    elif np.all(m2 != 0):
        mode = "full"
    else:
        mode = "generic"

    xT = np.ascontiguousarray(x[0].T).astype(bf)
    in_maps = []
    for c in range(NCORES):
        qs = slice(4 * c * 128, (4 * c + 4) * 128)
        ks = slice(c * 128, (c + 1) * 128)
        biasp = np.zeros((128, 6), np.float32)
        biasp[:, 0:4] = (bq[qs] * INV_SQRT_D).reshape(4, 128).T
        biasp[:, 4] = bk[ks]
        biasp[:, 5] = bv[ks]
        im = {
            "xT": xT,
            "wq": np.ascontiguousarray(wq[:, qs]).astype(bf),
            "wk": np.ascontiguousarray(wk[:, ks]).astype(bf),
            "wv": np.ascontiguousarray(wv[:, ks]).astype(bf),
            "wo": np.ascontiguousarray(wo[qs, :]).astype(bf),
            "biasp": biasp,
        }
        if mode == "generic":
            im["maskT"] = np.ascontiguousarray((m2 != 0).T).astype(bf)
        in_maps.append(im)

    nc = _get_prog(mode)
    res = run_bass_kernel_spmd(nc, in_maps, list(range(NCORES)), trace=TRACE)
    LAST_EXEC_NS = res.exec_time_ns
    LAST_RESULTS = res
    y = np.zeros((S, HIDDEN), np.float64)
    for r in res.results:
        y += r["y"].astype(np.float64)
    y = (y + bo.astype(np.float64)).astype(np.float32)
    return y[None]


# revision 9
# speedup vs baseline: 1.2245x; 1.2245x over previous
"""GQA attention layer for Trainium2, tensor-parallel over kv-heads on 8 NeuronCores.

Problem: x:(1,2048,2048) f32, causal mask; q/k/v/o projections with
NUM_HEADS=32, NUM_KV_HEADS=8, HEAD_DIM=128, GROUP=4.

Sharding: core c owns kv-head c and its 4 query heads (columns 4c*128..(4c+4)*128
of wq, rows of wo). Each core computes a partial y_c = attnout_c @ wo_c; the host
sums the 8 partials and adds bo.

Dataflow on each core (all "transposed" so no on-chip transposes of the big
probability matrix are ever needed):
  qT[d,i] = wq_c.T(h-major) accumulation:  matmul(lhsT=wq_kt, rhs=xT_kt)
  kT[d,j], vT[d,j] likewise;  v[j,d] via 16 PE transposes of vT.
  sT[j,i] = matmul(lhsT=kT_jtile, rhs=qT_chunk)   (contraction = head_dim, 1 mm)
  e = exp(sT * 1/sqrt(d))  on ACT (scale folded into qT drain), causal-masked
      with gpsimd.affine_select on the 4 diagonal j-tiles of each i-chunk.
  colsum[1,i] += ones.T @ e  (PE);  attnoutT[d,i] += v_jtile.T(=v) @ e (PE)
  recip = 1/colsum (DVE);  broadcast to 128 partitions with a k=1 PE matmul;
  aoT = avpsum * recip (DVE drain, bf16)
  y[i,hid] += aoT_head_tile.T @ wo_head  (4 head k-tiles), f32 out, DMA to HBM.

Causality: for i-chunk c (512 wide) only j-tiles 0..4c+3 are computed.
"""

import math

import numpy as np
import ml_dtypes

HIDDEN = 2048
HEAD_DIM = 128
NUM_HEADS = 32
NUM_KV = 8
GROUP = NUM_HEADS // NUM_KV
S = 2048
NCORES = 8
CH = 512                      # i-chunk width
NCH = S // CH                 # 4 i-chunks
KT = HIDDEN // 128            # 16 contraction tiles over hidden
NJT = S // 128                # 16 j-tiles
INV_SQRT_D = 1.0 / math.sqrt(HEAD_DIM)

# Module-level knobs for test.py (the grading harness uses the defaults).
TRACE = False
LAST_EXEC_NS = None
LAST_RESULTS = None

_PROG_CACHE = {}


def _build(mode):
    """mode: 'causal' (skip upper blocks, affine_select diag), 'full' (all-ones
    mask), 'generic' (multiplicative bf16 mask tiles from HBM)."""
    import concourse.bacc as bacc
    import concourse.tile as tile
    import concourse.mybir as mybir
    from concourse.masks import make_identity

    f32 = mybir.dt.float32
    bf16 = mybir.dt.bfloat16
    Ident = mybir.ActivationFunctionType.Identity
    Exp = mybir.ActivationFunctionType.Exp

    nc = bacc.Bacc(None, target_bir_lowering=False)

    xT_d = nc.dram_tensor("xT", [HIDDEN, S], bf16, kind="ExternalInput")
    wq_d = nc.dram_tensor("wq", [HIDDEN, GROUP * HEAD_DIM], bf16, kind="ExternalInput")
    wk_d = nc.dram_tensor("wk", [HIDDEN, HEAD_DIM], bf16, kind="ExternalInput")
    wv_d = nc.dram_tensor("wv", [HIDDEN, HEAD_DIM], bf16, kind="ExternalInput")
    wo_d = nc.dram_tensor("wo", [GROUP * HEAD_DIM, HIDDEN], bf16, kind="ExternalInput")
    bias_d = nc.dram_tensor("biasp", [128, 6], f32, kind="ExternalInput")
    if mode == "causal":
        ms_d = nc.dram_tensor("mstrip", [128, 896], bf16, kind="ExternalInput")
    if mode == "generic":
        mk_d = nc.dram_tensor("maskT", [S, S], bf16, kind="ExternalInput")
    y_d = nc.dram_tensor("y", [S, HIDDEN], f32, kind="ExternalOutput")

    def nblocks(c):
        return 4 * c + 4 if mode == "causal" else NJT

    with tile.TileContext(nc) as tc:
        with (
            tc.tile_pool(name="consts", bufs=1) as consts,
            tc.tile_pool(name="xw", bufs=1) as xw,
            tc.tile_pool(name="proj", bufs=1) as proj,
            tc.tile_pool(name="epool", bufs=6) as epool,
            tc.tile_pool(name="rpool", bufs=2) as rpool,
            tc.tile_pool(name="ypool", bufs=3) as ypool,
            tc.tile_pool(name="pp", bufs=2, space="PSUM") as pp,
            tc.tile_pool(name="spp", bufs=3, space="PSUM") as spp,
            tc.tile_pool(name="avp", bufs=2, space="PSUM") as avp,
            tc.tile_pool(name="csp", bufs=1, space="PSUM") as csp,
        ):
            # ---- constants ----
            ident = consts.tile([128, 128], bf16, tag="ident", name="ident")
            make_identity(nc, ident)
            ones_col = consts.tile([128, 1], bf16, tag="ones_col", name="ones_col")
            nc.vector.memset(ones_col, 1.0)
            ones_row = consts.tile([1, 128], f32, tag="ones_row", name="ones_row")
            nc.vector.memset(ones_row, 1.0)
            bias_sb = consts.tile([128, 6], f32, tag="bias", name="bias_sb")
            nc.sync.dma_start(out=bias_sb, in_=bias_d[:, :])
            if mode == "causal":
                mstrip = consts.tile([128, 896], bf16, tag="mstrip", name="mstrip")
                nc.sync.dma_start(out=mstrip, in_=ms_d[:, :])

            # ---- input loads (k-tile-major so first proj block starts early) ----
            x_sb, wk_sb, wv_sb, wq_sb, wo_sb = [], [], [], [], []
            for kt in range(KT):
                wkt = xw.tile([128, HEAD_DIM], bf16, tag=f"wk{kt}", name=f"wk{kt}")
                nc.sync.dma_start(out=wkt, in_=wk_d[kt * 128:(kt + 1) * 128, :])
                wk_sb.append(wkt)
                xt = xw.tile([128, S], bf16, tag=f"x{kt}", name=f"x{kt}")
                nc.sync.dma_start(out=xt, in_=xT_d[kt * 128:(kt + 1) * 128, :])
                x_sb.append(xt)
            for kt in range(KT):
                wvt = xw.tile([128, HEAD_DIM], bf16, tag=f"wv{kt}", name=f"wv{kt}")
                nc.sync.dma_start(out=wvt, in_=wv_d[kt * 128:(kt + 1) * 128, :])
                wv_sb.append(wvt)
            for kt in range(KT):
                wqt = xw.tile([128, GROUP * HEAD_DIM], bf16, tag=f"wq{kt}", name=f"wq{kt}")
                nc.sync.dma_start(out=wqt, in_=wq_d[kt * 128:(kt + 1) * 128, :])
                wq_sb.append(wqt)
            for h in range(GROUP):
                wot = xw.tile([128, HIDDEN], bf16, tag=f"wo{h}", name=f"wo{h}")
                nc.sync.dma_start(out=wot, in_=wo_d[h * 128:(h + 1) * 128, :])
                wo_sb.append(wot)

            # ---- K projection: kT[d, j] per j-chunk ----
            kT_c = []
            for c in range(NCH):
                ps = pp.tile([128, CH], f32, tag="pp", name=f"psk{c}")
                for kt in range(KT):
                    nc.tensor.matmul(ps, lhsT=wk_sb[kt],
                                     rhs=x_sb[kt][:, c * CH:(c + 1) * CH],
                                     start=(kt == 0), stop=(kt == KT - 1))
                kt_t = proj.tile([128, CH], bf16, tag=f"kT{c}", name=f"kT{c}")
                nc.scalar.activation(kt_t, ps, Ident, bias=bias_sb[:, 4:5])
                kT_c.append(kt_t)

            # ---- V projection (vT) + PE transpose to v[j, d] ----
            vT_c = []
            for c in range(NCH):
                ps = pp.tile([128, CH], f32, tag="pp", name=f"psv{c}")
                for kt in range(KT):
                    nc.tensor.matmul(ps, lhsT=wv_sb[kt],
                                     rhs=x_sb[kt][:, c * CH:(c + 1) * CH],
                                     start=(kt == 0), stop=(kt == KT - 1))
                vt_t = proj.tile([128, CH], bf16, tag=f"vT{c}", name=f"vT{c}")
                nc.scalar.activation(vt_t, ps, Ident, bias=bias_sb[:, 5:6])
                vT_c.append(vt_t)
            v_sb = []
            for b in range(NJT):
                tp = spp.tile([128, 128], bf16, tag="s", name=f"tp{b}")
                nc.tensor.transpose(
                    tp, vT_c[b // 4][:, (b % 4) * 128:(b % 4 + 1) * 128], ident)
                vt = proj.tile([128, 128], bf16, tag=f"v{b}", name=f"v{b}")
                nc.vector.tensor_copy(vt, tp)
                v_sb.append(vt)

            # ---- Q projection: qT[(h, c)]; fold 1/sqrt(d) + scaled bias ----
            qT = {}
            for c in range(NCH):
                for h in range(GROUP):
                    ps = pp.tile([128, CH], f32, tag="pp", name=f"psq{h}_{c}")
                    for kt in range(KT):
                        nc.tensor.matmul(
                            ps, lhsT=wq_sb[kt][:, h * 128:(h + 1) * 128],
                            rhs=x_sb[kt][:, c * CH:(c + 1) * CH],
                            start=(kt == 0), stop=(kt == KT - 1))
                    qt_t = proj.tile([128, CH], bf16, tag=f"q{h}_{c}", name=f"q{h}_{c}")
                    nc.scalar.activation(qt_t, ps, Ident,
                                         bias=bias_sb[:, h:h + 1], scale=INV_SQRT_D)
                    qT[(h, c)] = qt_t

            # ---- generic-mode mask tiles (per chunk, shared across heads) ----
            mask_sb = {}

            # ---- attention + output projection, chunk-major ----
            aoT = {}
            for c in range(NCH):
                nb = nblocks(c)
                if mode == "generic":
                    for b in range(nb):
                        mt = proj.tile([128, CH], bf16, tag=f"m{b}", name=f"m{b}_{c}")
                        nc.sync.dma_start(
                            out=mt,
                            in_=mk_d[b * 128:(b + 1) * 128, c * CH:(c + 1) * CH])
                        mask_sb[b] = mt
                for h in range(GROUP):
                    av = avp.tile([128, CH], f32, tag="av", name=f"av{h}_{c}")
                    cs = csp.tile([1, CH], f32, tag="cs", name=f"cs{h}_{c}")
                    e_tiles = {}

                    def tail(b, nb=nb, h=h, c=c, av=av, cs=cs, e_tiles=e_tiles):
                        e = e_tiles.pop(b)
                        nc.tensor.matmul(cs, lhsT=ones_col, rhs=e,
                                         start=(b == 0), stop=(b == nb - 1),
                                         skip_group_check=True)
                        nc.tensor.matmul(av, lhsT=v_sb[b], rhs=e,
                                         start=(b == 0), stop=(b == nb - 1),
                                         skip_group_check=True)

                    LAG = 2
                    for b in range(nb):
                        sp_t = spp.tile([128, CH], f32, tag="s", name=f"s{h}_{c}_{b}")
                        nc.tensor.matmul(
                            sp_t, lhsT=kT_c[b // 4][:, (b % 4) * 128:(b % 4 + 1) * 128],
                            rhs=qT[(h, c)], start=True, stop=True)
                        e = epool.tile([128, CH], bf16, tag="e", name=f"e{h}_{c}_{b}")
                        nc.scalar.activation(e, sp_t, Exp)
                        if mode == "causal" and b >= 4 * c:
                            dd = b - 4 * c
                            st = 384 - 128 * dd
                            nc.vector.tensor_mul(e, e, mstrip[:, st:st + CH])
                        elif mode == "generic":
                            nc.vector.tensor_mul(e, e, mask_sb[b])
                        e_tiles[b] = e
                        if b >= LAG:
                            tail(b - LAG)
                    for b in range(max(nb - LAG, 0), nb):
                        tail(b)
                    recip = rpool.tile([1, CH], f32, tag="recip", name=f"rc{h}_{c}")
                    nc.vector.reciprocal_approx_fast(recip, cs)
                    rb_ps = spp.tile([128, CH], f32, tag="s", name=f"rbp{h}_{c}")
                    nc.tensor.matmul(rb_ps, lhsT=ones_row, rhs=recip,
                                     start=True, stop=True)
                    rb = rpool.tile([128, CH], f32, tag="rb", name=f"rb{h}_{c}")
                    nc.vector.tensor_copy(rb, rb_ps)
                    ao = proj.tile([128, CH], bf16, tag=f"ao{h}_{c}", name=f"ao{h}_{c}")
                    nc.vector.tensor_mul(ao, av, rb)
                    aoT[(h, c)] = ao
                # y projection for this chunk
                for it in range(CH // 128):
                    for nh in range(NCH):
                        yp = pp.tile([128, CH], f32, tag="pp", name=f"yp{c}_{it}_{nh}")
                        for h in range(GROUP):
                            nc.tensor.matmul(
                                yp, lhsT=aoT[(h, c)][:, it * 128:(it + 1) * 128],
                                rhs=wo_sb[h][:, nh * CH:(nh + 1) * CH],
                                start=(h == 0), stop=(h == GROUP - 1))
                        ysb = ypool.tile([128, CH], f32, tag="y", name=f"y{c}_{it}_{nh}")
                        nc.vector.tensor_copy(ysb, yp)
                        nc.sync.dma_start(
                            out=y_d[c * CH + it * 128: c * CH + (it + 1) * 128,
                                    nh * CH:(nh + 1) * CH],
                            in_=ysb)
    nc.finalize()
    return nc


def _get_prog(mode):
    if mode not in _PROG_CACHE:
        _PROG_CACHE[mode] = _build(mode)
    return _PROG_CACHE[mode]


def kernel(x, mask, wq, bq, wk, bk, wv, bv, wo, bo):
    global LAST_EXEC_NS, LAST_RESULTS
    from concourse.bass_utils import run_bass_kernel_spmd

    bf = ml_dtypes.bfloat16
    x = np.asarray(x, dtype=np.float32)
    mask = np.asarray(mask)
    wq = np.asarray(wq, dtype=np.float32)
    bq = np.asarray(bq, dtype=np.float32)
    wk = np.asarray(wk, dtype=np.float32)
    bk = np.asarray(bk, dtype=np.float32)
    wv = np.asarray(wv, dtype=np.float32)
    bv = np.asarray(bv, dtype=np.float32)
    wo = np.asarray(wo, dtype=np.float32)
    bo = np.asarray(bo, dtype=np.float32)

    m2 = mask[0, 0]
    if np.array_equal(m2 != 0, np.tril(np.ones((S, S), dtype=bool))):
        mode = "causal"
    elif np.all(m2 != 0):
        mode = "full"
    else:
        mode = "generic"

    xT = np.ascontiguousarray(x[0].T).astype(bf)
    if mode == "causal":
        g = np.arange(896)[None, :]
        p = np.arange(128)[:, None]
        mstrip = (g - p >= 384).astype(bf)
    in_maps = []
    for c in range(NCORES):
        qs = slice(4 * c * 128, (4 * c + 4) * 128)
        ks = slice(c * 128, (c + 1) * 128)
        biasp = np.zeros((128, 6), np.float32)
        biasp[:, 0:4] = (bq[qs] * INV_SQRT_D).reshape(4, 128).T
        biasp[:, 4] = bk[ks]
        biasp[:, 5] = bv[ks]
        im = {
            "xT": xT,
            "wq": np.ascontiguousarray(wq[:, qs]).astype(bf),
            "wk": np.ascontiguousarray(wk[:, ks]).astype(bf),
            "wv": np.ascontiguousarray(wv[:, ks]).astype(bf),
            "wo": np.ascontiguousarray(wo[qs, :]).astype(bf),
            "biasp": biasp,
        }
        if mode == "causal":
            im["mstrip"] = mstrip
        if mode == "generic":
            im["maskT"] = np.ascontiguousarray((m2 != 0).T).astype(bf)
        in_maps.append(im)

    nc = _get_prog(mode)
    res = run_bass_kernel_spmd(nc, in_maps, list(range(NCORES)), trace=TRACE)
    LAST_EXEC_NS = res.exec_time_ns
    LAST_RESULTS = res
    y = np.zeros((S, HIDDEN), np.float64)
    for r in res.results:
        y += r["y"].astype(np.float64)
    y = (y + bo.astype(np.float64)).astype(np.float32)
    return y[None]
